# revision 1
# baseline (speedup 1.0000x reference)
"""AEM attention-pooling kernel for 8 Trainium2 NeuronCores.

Strategy: data-parallel over batch (512 rows/core), embedding tables
replicated, zero collectives.

Gather engine: dma_gather (int16 indices) with the tables split into
32768-row windows. Host-side slot assignment places each batch row's
items in its own SBUF partition: for window w, the gather index list is
arranged so list position k = c*128 + p holds the c-th in-window item of
the batch row assigned to partition p (dma_gather writes position k to
partition k%128, free chunk k//128). Slots beyond a row's in-window
count point at a pad row and are masked out of the softmax. The word
table gets an explicit zero row per window so word pads vanish from the
mean without masking.

Math per row b (red_w folded into the attention vector):
    qm = mean_j word_emb[qw[b,j]];  q = tanh(qp_w @ qm + qp_b)
    v  = sum_h red_w[h] * tanh(aq_w_h @ q + aq_b_h)
    s_c = bought[b,c,:] . v;  w = softmax_masked(s);  user = sum w_c bought_c
    out = 0.5*(q + user)

Engines: 6 consolidated dma_gathers on GPSIMD (one per table window,
spread over 4 SWDGE queues, words first so the MLP overlaps item
gathers); scores/weighted-rows as blockwise broadcast-multiplies +
reduces on VectorE; word-mean and attention-weighted sums as N=512
batched identity-matmul PSUM accumulations on TensorE; tanh/exp on
ScalarE. Indices are int16 (dma_gather requirement), hence the
32768-row table windows.
"""

import numpy as np

try:
    import ml_dtypes

    _BF16 = ml_dtypes.bfloat16
except ImportError:  # pragma: no cover
    _BF16 = None

B, I, QW = 4096, 50, 20
WORD_NUM, ITEM_NUM, E, H = 50000, 100000, 128, 8
N_CORES = 8
B_CORE = B // N_CORES
P = 128
WIN = 32768
ITEM_WINS = 4   # ceil(100000 / 32768)
WORD_WINS = 2   # padded word table: window bases 0 and 32768

ITEM_WIN_BASES = [0, WIN, 2 * WIN, 3 * WIN]
WORD_WIN_BASES = [0, WIN]
WORD_V_PADDED = WORD_NUM + 2  # zero row at 0 and at 32768


def plan_layout(ubi, qw_arr):
    """Host planning: per-core batch-row assignment + shared chunk caps.

    Returns perm[core] ([512] original batch row for slot tile*128+p) and
    per-tile-slot window chunk caps shared across cores (SPMD needs one
    graph shape for all cores).
    """
    # GLOBAL sort of all batch rows by item window profile, then deal rows so
    # that tile-slot t of every core draws from the same profile segment --
    # the shared (max-over-cores) caps stay tight.
    ic_all = np.stack(
        [((ubi >= w * WIN) & (ubi < (w + 1) * WIN)).sum(1) for w in range(ITEM_WINS)],
        axis=1,
    )
    order_all = np.lexsort((ic_all[:, 2], ic_all[:, 1], ic_all[:, 0]))
    n_tiles_ = B_CORE // P
    perm = np.zeros((N_CORES, B_CORE), np.int64)
    item_cnt = np.zeros((N_CORES, B_CORE, ITEM_WINS), np.int64)
    word_cnt = np.zeros((N_CORES, B_CORE, WORD_WINS), np.int64)
    for t in range(n_tiles_):
        seg = order_all[t * P * N_CORES : (t + 1) * P * N_CORES]
        for c in range(N_CORES):
            rows = seg[c * P : (c + 1) * P]
            perm[c, t * P : (t + 1) * P] = rows
            item_cnt[c, t * P : (t + 1) * P] = ic_all[rows]
            wc0 = (qw_arr[rows] < (WIN - 1)).sum(1)
            word_cnt[c, t * P : (t + 1) * P] = np.stack([wc0, QW - wc0], axis=1)
    n_tiles = B_CORE // P
    item_caps = np.zeros((n_tiles, ITEM_WINS), np.int64)
    word_caps = np.zeros((n_tiles, WORD_WINS), np.int64)
    for t in range(n_tiles):
        item_caps[t] = item_cnt[:, t * P : (t + 1) * P].max(axis=(0, 1))
        word_caps[t] = word_cnt[:, t * P : (t + 1) * P].max(axis=(0, 1))
    return perm, item_caps, word_caps


def _wrap16(lst):
    """dma_gather index wrapping: position g -> (partition g%16, slot g//16),
    replicated to all 8 16-partition groups."""
    n = lst.shape[0]
    assert n % 16 == 0
    w = lst.reshape(n // 16, 16).T.astype(np.int16)  # [16, n/16]
    return np.tile(w, (8, 1))  # [128, n/16]


def build_host_inputs(inputs):
    """All host prep: planning, index lists, masks, tables, weights."""
    ubi = np.asarray(inputs["user_bought_items"]).astype(np.int64)
    qw_arr = np.asarray(inputs["query_words"]).astype(np.int64)
    masks_in = np.asarray(inputs["user_bought_masks"], dtype=np.float32).reshape(B, I)
    word_emb = np.asarray(inputs["word_emb"], dtype=np.float32)
    item_emb = np.asarray(inputs["item_emb"], dtype=np.float32)
    qp_w = np.asarray(inputs["qp_w"], dtype=np.float32)
    qp_b = np.asarray(inputs["qp_b"], dtype=np.float32)
    aq_w = np.asarray(inputs["aq_w"], dtype=np.float32)
    aq_b = np.asarray(inputs["aq_b"], dtype=np.float32)
    red_w = np.asarray(inputs["red_w"], dtype=np.float32)

    perm, item_caps, word_caps = plan_layout(ubi, qw_arr)
    n_tiles = B_CORE // P
    CH = [int(item_caps[t].sum()) for t in range(n_tiles)]
    CW = [int(word_caps[t].sum()) for t in range(n_tiles)]

    # padded word table: zero rows at 0 and 32768
    # remap: old r < 32767 -> r+1 (window 0); old r >= 32767 -> r+2 (window 1)
    word_tab = np.zeros((WORD_V_PADDED, E), np.float32)
    word_tab[1:WIN] = word_emb[: WIN - 1]
    word_tab[WIN + 1 :] = word_emb[WIN - 1 :]
    word_tab_b16 = np.ascontiguousarray(word_tab.astype(_BF16))
    item_tab_b16 = np.ascontiguousarray(item_emb.astype(_BF16))

    # index lists grouped per WINDOW across all tiles (one gather per window);
    # within a window, tile t's block occupies chunks [sum_t'<t caps[t'][w], ...)
    item_idx_packed = []
    word_idx_packed = []
    mask_packed = []
    for c in range(N_CORES):
        ilists = [[] for _ in range(ITEM_WINS)]   # per window: chunk lists per tile
        wlists = [[] for _ in range(WORD_WINS)]
        mparts = []
        for t in range(n_tiles):
            rows = perm[c, t * P : (t + 1) * P]
            sub_ubi = ubi[rows]
            sub_qw = qw_arr[rows]
            sub_mask = masks_in[rows]
            mtile = np.zeros((P, CH[t]), np.float32)
            off = 0
            for w in range(ITEM_WINS):
                cap = int(item_caps[t, w])
                lst = np.zeros(cap * P, np.int64)  # pad 0 = window base row
                for p in range(P):
                    iw = np.where((sub_ubi[p] >= w * WIN) & (sub_ubi[p] < (w + 1) * WIN))[0]
                    loc = sub_ubi[p, iw] - w * WIN
                    for c2, (li, ii) in enumerate(zip(loc, iw)):
                        lst[c2 * P + p] = li
                        mtile[p, off + c2] = sub_mask[p, ii]
                ilists[w].append(lst)
                off += cap
            for w in range(WORD_WINS):
                cap = int(word_caps[t, w])
                lst = np.zeros(cap * P, np.int64)  # pad 0 = zero row
                for p in range(P):
                    if w == 0:
                        jw = np.where(sub_qw[p] < (WIN - 1))[0]
                        loc = sub_qw[p, jw] + 1
                    else:
                        jw = np.where(sub_qw[p] >= (WIN - 1))[0]
                        loc = sub_qw[p, jw] + 2 - WIN
                    for c2, li in enumerate(loc):
                        lst[c2 * P + p] = li
                wlists[w].append(lst)
            mparts.append(mtile)
        iparts = [_wrap16(np.concatenate(ilists[w])) for w in range(ITEM_WINS)]
        wparts = [_wrap16(np.concatenate(wlists[w])) for w in range(WORD_WINS)]
        item_idx_packed.append(np.concatenate(iparts, axis=1))
        word_idx_packed.append(np.concatenate(wparts, axis=1))
        mask_packed.append(np.concatenate(mparts, axis=1))

    qp_wT = np.ascontiguousarray(qp_w.T.astype(_BF16))
    aq_wT = np.ascontiguousarray(aq_w.T.astype(_BF16))
    qp_b_c = np.ascontiguousarray(qp_b.reshape(E, 1))
    aq_bT = np.ascontiguousarray(aq_b.reshape(H, E).T)
    red_w_r = np.ascontiguousarray(red_w.reshape(1, H))

    in_maps = []
    for c in range(N_CORES):
        in_maps.append(
            {
                "item_idx": item_idx_packed[c],
                "word_idx": word_idx_packed[c],
                "maskp": mask_packed[c],
                "word_tab": word_tab_b16,
                "item_tab": item_tab_b16,
                "qp_wT": qp_wT,
                "qp_b": qp_b_c,
                "aq_wT": aq_wT,
                "aq_bT": aq_bT,
                "red_w": red_w_r,
            }
        )
    shapes = {
        "item_caps": [[int(x) for x in row] for row in item_caps],
        "word_caps": [[int(x) for x in row] for row in word_caps],
        "CH": CH,
        "CW": CW,
        "item_idx_w": int(item_idx_packed[0].shape[1]),
        "word_idx_w": int(word_idx_packed[0].shape[1]),
        "mask_w": int(mask_packed[0].shape[1]),
    }
    return in_maps, shapes, perm


def build_nc(shapes):
    import concourse.bacc as bacc
    import concourse.mybir as mybir
    from concourse.masks import make_identity
    from concourse.tile import TileContext

    f32 = mybir.dt.float32
    bf16 = mybir.dt.bfloat16
    i16 = mybir.dt.int16
    AF = mybir.ActivationFunctionType
    OP = mybir.AluOpType

    item_caps = shapes["item_caps"]
    word_caps = shapes["word_caps"]
    CH = shapes["CH"]
    CW = shapes["CW"]
    n_tiles = len(CH)
    ITOT = [sum(item_caps[t][w] for t in range(n_tiles)) for w in range(ITEM_WINS)]
    WTOT = [sum(word_caps[t][w] for t in range(n_tiles)) for w in range(WORD_WINS)]
    SCH = sum(ITOT)
    SCW = sum(WTOT)
    # global chunk offset of tile t's window-w block in bought_all / wg_all
    IW_START = [sum(ITOT[:w]) for w in range(ITEM_WINS)]
    WW_START = [sum(WTOT[:w]) for w in range(WORD_WINS)]

    def gcol_item(t, w):
        return IW_START[w] + sum(item_caps[tt][w] for tt in range(t))

    def gcol_word(t, w):
        return WW_START[w] + sum(word_caps[tt][w] for tt in range(t))

    nc = bacc.Bacc("TRN2", num_swdge_queues=4)

    item_idx = nc.dram_tensor("item_idx", [P, shapes["item_idx_w"]], i16, kind="ExternalInput")
    word_idx = nc.dram_tensor("word_idx", [P, shapes["word_idx_w"]], i16, kind="ExternalInput")
    maskp = nc.dram_tensor("maskp", [P, shapes["mask_w"]], f32, kind="ExternalInput")
    word_tab = nc.dram_tensor("word_tab", [WORD_V_PADDED, E], bf16, kind="ExternalInput")
    item_tab = nc.dram_tensor("item_tab", [ITEM_NUM, E], bf16, kind="ExternalInput")
    qp_wT = nc.dram_tensor("qp_wT", [E, E], bf16, kind="ExternalInput")
    qp_b = nc.dram_tensor("qp_b", [E, 1], f32, kind="ExternalInput")
    aq_wT = nc.dram_tensor("aq_wT", [E, E * H], bf16, kind="ExternalInput")
    aq_bT = nc.dram_tensor("aq_bT", [E, H], f32, kind="ExternalInput")
    red_w = nc.dram_tensor("red_w", [1, H], f32, kind="ExternalInput")
    out = nc.dram_tensor("out", [B_CORE, E], f32, kind="ExternalOutput")

    with TileContext(nc) as tc:
        with (
            tc.tile_pool(name="const", bufs=1) as cpool,
            tc.tile_pool(name="big", bufs=1) as big,
            tc.tile_pool(name="mid", bufs=2) as mid,
            tc.tile_pool(name="small", bufs=3) as small,
            tc.tile_pool(name="psum_acc", bufs=1, space="PSUM") as pacc,
            tc.tile_pool(name="psum_tr", bufs=2, space="PSUM") as ptr,
        ):
            ident = cpool.tile([P, P], bf16, tag="ident")
            make_identity(nc, ident[:])
            ident_f = cpool.tile([P, P], f32, tag="ident_f")
            make_identity(nc, ident_f[:])

            qp_wT_sb = cpool.tile([E, E], bf16, tag="qp_wT_sb")
            nc.sync.dma_start(out=qp_wT_sb[:], in_=qp_wT[:, :])
            aq_wT_sb = cpool.tile([E, E * H], bf16, tag="aq_wT_sb")
            nc.sync.dma_start(out=aq_wT_sb[:], in_=aq_wT[:, :])
            qp_b_sb = cpool.tile([E, 1], f32, tag="qp_b_sb")
            nc.sync.dma_start(out=qp_b_sb[:], in_=qp_b[:, :])
            aq_bT_sb = cpool.tile([E, H], f32, tag="aq_bT_sb")
            nc.sync.dma_start(out=aq_bT_sb[:], in_=aq_bT[:, :])
            red_w_sb = cpool.tile([1, H], f32, tag="red_w_sb")
            nc.sync.dma_start(out=red_w_sb[:], in_=red_w[:, :])

            ones_col = cpool.tile([1, P], f32, tag="ones_col")
            nc.vector.memset(ones_col[:], 1.0)
            rw_psum = ptr.tile([P, H], f32, tag="mm", space="PSUM")
            nc.tensor.matmul(out=rw_psum[:], lhsT=ones_col[:], rhs=red_w_sb[:], start=True, stop=True)
            rw_bcast = cpool.tile([P, H], f32, tag="rw_bcast")
            nc.scalar.copy(out=rw_bcast[:], in_=rw_psum[:])

            scaledI = cpool.tile([P, H * P], bf16, tag="scaledI")
            for h in range(H):
                nc.vector.tensor_scalar_mul(
                    out=scaledI[:, h * P : (h + 1) * P], in0=ident[:],
                    scalar1=rw_bcast[:, h : h + 1],
                )

            # ---- all index tiles + 6 consolidated gathers ----
            iidx_sb = cpool.tile([P, shapes["item_idx_w"]], i16, tag="iidx_sb")
            nc.sync.dma_start(out=iidx_sb[:], in_=item_idx[:, :])
            widx_sb = cpool.tile([P, shapes["word_idx_w"]], i16, tag="widx_sb")
            nc.sync.dma_start(out=widx_sb[:], in_=word_idx[:, :])

            bought_all = cpool.tile([P, SCH, E], bf16, tag="bought_all")
            wg_all = cpool.tile([P, SCW, E], bf16, tag="wg_all")
            # words first (the MLP chain consumes them and can overlap the
            # item gathers); spread queues so big gathers run concurrently
            off = 0
            wqueues = [0, 1]
            for w in range(WORD_WINS):
                tot = WTOT[w]
                if tot == 0:
                    continue
                n = tot * P
                src = word_tab[WORD_WIN_BASES[w] :, :] if w else word_tab[:, :]
                nc.gpsimd.dma_gather(
                    out_ap=wg_all[:, off : off + tot, :], in_ap=src,
                    idxs_ap=widx_sb[:, off * 8 : (off + tot) * 8],
                    num_idxs=n, num_idxs_reg=n, elem_size=E,
                    single_packet=False, queue_num=wqueues[w % len(wqueues)],
                )
                off += tot
            off = 0
            iqueues = [2, 3, 1, 0]
            for w in range(ITEM_WINS):
                tot = ITOT[w]
                if tot == 0:
                    continue
                n = tot * P
                src = item_tab[ITEM_WIN_BASES[w] :, :] if w else item_tab[:, :]
                nc.gpsimd.dma_gather(
                    out_ap=bought_all[:, off : off + tot, :], in_ap=src,
                    idxs_ap=iidx_sb[:, off * 8 : (off + tot) * 8],
                    num_idxs=n, num_idxs_reg=n, elem_size=E,
                    single_packet=False, queue_num=iqueues[w],
                )
                off += tot

            m_off = 0
            for t in range(n_tiles):
                ch = CH[t]
                cw = CW[t]
                b0 = t * P

                mask_sb = small.tile([P, ch], f32, tag="mask_sb")
                nc.sync.dma_start(out=mask_sb[:], in_=maskp[:, m_off : m_off + ch])
                m_off += ch

                # ---- word mean via N=512 batched identity-matmuls ----
                qm4_psum = pacc.tile([P, 4 * E], f32, tag="qm4", space="PSUM")
                wblocks = []
                for w in range(WORD_WINS):
                    g0 = gcol_word(t, w)
                    wblocks.append((g0, int(word_caps[t][w])))
                ngroups = []
                for g0, cap in wblocks:
                    c = 0
                    while c < cap:
                        r = min(4, cap - c)
                        ngroups.append((g0 + c, r))
                        c += r
                for gi, (c0, r) in enumerate(ngroups):
                    nc.tensor.matmul(
                        out=qm4_psum[:, : r * E],
                        lhsT=ident[:],
                        rhs=wg_all[:, c0 : c0 + r, :].rearrange("p c e -> p (c e)"),
                        start=(gi == 0), stop=(gi == len(ngroups) - 1),
                    )
                qm_c1 = small.tile([P, E], f32, tag="qm_c1")
                nc.scalar.copy(out=qm_c1[:], in_=qm4_psum[:, E : 2 * E])
                qm_c3 = small.tile([P, E], f32, tag="qm_c3")
                nc.scalar.copy(out=qm_c3[:], in_=qm4_psum[:, 3 * E : 4 * E])
                qm_a = small.tile([P, E], f32, tag="qm_a")
                nc.vector.tensor_tensor(out=qm_a[:], in0=qm4_psum[:, 0:E], in1=qm_c1[:], op=OP.add)
                qm_b = small.tile([P, E], f32, tag="qm_b")
                nc.vector.tensor_tensor(out=qm_b[:], in0=qm4_psum[:, 2 * E : 3 * E], in1=qm_c3[:], op=OP.add)
                qm_sb = small.tile([P, E], f32, tag="qm_sb")
                nc.vector.tensor_tensor(out=qm_sb[:], in0=qm_a[:], in1=qm_b[:], op=OP.add)

                qmT_psum = ptr.tile([E, P], f32, tag="mmt", space="PSUM")
                nc.tensor.transpose(out=qmT_psum[:], in_=qm_sb[:], identity=ident_f[:])
                qmT_sb = small.tile([E, P], bf16, tag="qmT_sb")
                nc.scalar.mul(out=qmT_sb[:], in_=qmT_psum[:], mul=1.0 / QW)

                mm1_psum = ptr.tile([E, P], f32, tag="mm", space="PSUM")
                nc.tensor.matmul(out=mm1_psum[:], lhsT=qp_wT_sb[:], rhs=qmT_sb[:], start=True, stop=True)
                qT_f = small.tile([E, P], f32, tag="qT_f")
                nc.scalar.activation(out=qT_f[:], in_=mm1_psum[:], func=AF.Tanh, bias=qp_b_sb[:, 0:1])
                qT_sb = small.tile([E, P], bf16, tag="qT_sb")
                nc.vector.tensor_copy(out=qT_sb[:], in_=qT_f[:])

                q_psum = ptr.tile([P, E], f32, tag="mmt", space="PSUM")
                nc.tensor.transpose(out=q_psum[:], in_=qT_f[:], identity=ident_f[:])
                qhalf_sb = small.tile([P, E], f32, tag="qhalf_sb")
                nc.scalar.mul(out=qhalf_sb[:], in_=q_psum[:], mul=0.5)

                vT_psum = pacc.tile([E, P], f32, tag="vT", space="PSUM")
                for h in range(H):
                    mm2_psum = ptr.tile([E, P], f32, tag="mm", space="PSUM")
                    nc.tensor.matmul(
                        out=mm2_psum[:], lhsT=aq_wT_sb[:, h * E : (h + 1) * E],
                        rhs=qT_sb[:], start=True, stop=True,
                    )
                    t_h = small.tile([E, P], bf16, tag="t_h")
                    nc.scalar.activation(
                        out=t_h[:], in_=mm2_psum[:], func=AF.Tanh, bias=aq_bT_sb[:, h : h + 1]
                    )
                    nc.tensor.matmul(
                        out=vT_psum[:], lhsT=scaledI[:, h * P : (h + 1) * P],
                        rhs=t_h[:], start=(h == 0), stop=(h == H - 1),
                    )
                vT_sb = small.tile([E, P], f32, tag="vT_sb")
                nc.scalar.copy(out=vT_sb[:], in_=vT_psum[:])
                v_psum = ptr.tile([P, E], f32, tag="mmt", space="PSUM")
                nc.tensor.transpose(out=v_psum[:], in_=vT_sb[:], identity=ident_f[:])
                v_sb = small.tile([P, E], bf16, tag="v_sb")
                nc.scalar.copy(out=v_sb[:], in_=v_psum[:])

                iblocks = []
                for w in range(ITEM_WINS):
                    cap = int(item_caps[t][w])
                    if cap:
                        iblocks.append((gcol_item(t, w), cap))

                # ---- scores: blockwise bought * v-broadcast, reduce over E ----
                scores = small.tile([P, ch], f32, tag="scores")
                loff = 0
                for c0, cap in iblocks:
                    prodS = mid.tile([P, cap, E], bf16, tag="prodS")
                    nc.vector.tensor_tensor(
                        out=prodS[:, :, :],
                        in0=bought_all[:, c0 : c0 + cap, :],
                        in1=v_sb[:, None, :].to_broadcast([P, cap, E]),
                        op=OP.mult,
                    )
                    nc.vector.tensor_reduce(
                        out=scores[:, loff : loff + cap], in_=prodS[:, :, :],
                        axis=mybir.AxisListType.X, op=OP.add,
                    )
                    loff += cap

                negmax = small.tile([P, 1], f32, tag="negmax")
                nc.vector.reduce_max(out=negmax[:], in_=scores[:], axis=mybir.AxisListType.X, negate=True)
                att = small.tile([P, ch], f32, tag="att")
                nc.scalar.activation(out=att[:], in_=scores[:], func=AF.Exp, bias=negmax[:, 0:1])
                attm = small.tile([P, ch], f32, tag="attm")
                nc.vector.tensor_tensor(out=attm[:], in0=att[:], in1=mask_sb[:], op=OP.mult)
                denom = small.tile([P, 1], f32, tag="denom")
                nc.vector.reduce_sum(out=denom[:], in_=attm[:], axis=mybir.AxisListType.X)
                lt01 = small.tile([P, 1], f32, tag="lt01")
                nc.vector.tensor_scalar(out=lt01[:], in0=denom[:], scalar1=1e-7, scalar2=None, op0=OP.is_lt)
                denom2 = small.tile([P, 1], f32, tag="denom2")
                nc.vector.tensor_tensor(out=denom2[:], in0=denom[:], in1=lt01[:], op=OP.add)
                nc.vector.tensor_scalar_mul(out=denom2[:], in0=denom2[:], scalar1=2.0)
                rcp = small.tile([P, 1], f32, tag="rcp")
                nc.vector.reciprocal(out=rcp[:], in_=denom2[:])

                # attm as bf16 for the big product
                attm_b = small.tile([P, ch], bf16, tag="attm_b")
                nc.vector.tensor_copy(out=attm_b[:], in_=attm[:])

                # ---- user: blockwise bought * attm-broadcast + batched matmuls ----
                prodU = big.tile([P, ch, E], bf16, tag="prodU")
                loff = 0
                for c0, cap in iblocks:
                    nc.vector.tensor_tensor(
                        out=prodU[:, loff : loff + cap, :],
                        in0=bought_all[:, c0 : c0 + cap, :],
                        in1=attm_b[:, loff : loff + cap, None].to_broadcast([P, cap, E]),
                        op=OP.mult,
                    )
                    loff += cap
                u4_psum = pacc.tile([P, 4 * E], f32, tag="u4", space="PSUM")
                ngroups = (ch + 3) // 4
                for g in range(ngroups):
                    c0 = g * 4
                    r = min(4, ch - c0)
                    nc.tensor.matmul(
                        out=u4_psum[:, : r * E],
                        lhsT=ident[:],
                        rhs=prodU[:, c0 : c0 + r, :].rearrange("p c e -> p (c e)"),
                        start=(g == 0), stop=(g == ngroups - 1),
                    )
                u_c1 = small.tile([P, E], f32, tag="u_c1")
                nc.scalar.copy(out=u_c1[:], in_=u4_psum[:, E : 2 * E])
                u_c3 = small.tile([P, E], f32, tag="u_c3")
                nc.scalar.copy(out=u_c3[:], in_=u4_psum[:, 3 * E : 4 * E])
                u_a = small.tile([P, E], f32, tag="u_a")
                nc.vector.tensor_tensor(out=u_a[:], in0=u4_psum[:, 0:E], in1=u_c1[:], op=OP.add)
                u_b = small.tile([P, E], f32, tag="u_b")
                nc.vector.tensor_tensor(out=u_b[:], in0=u4_psum[:, 2 * E : 3 * E], in1=u_c3[:], op=OP.add)
                user_sb = small.tile([P, E], f32, tag="user_sb")
                nc.vector.tensor_tensor(out=user_sb[:], in0=u_a[:], in1=u_b[:], op=OP.add)

                out_sb = small.tile([P, E], f32, tag="out_sb")
                nc.vector.scalar_tensor_tensor(
                    out=out_sb[:], in0=user_sb[:], scalar=rcp[:, 0:1],
                    in1=qhalf_sb[:], op0=OP.mult, op1=OP.add,
                )
                nc.sync.dma_start(out=out[b0 : b0 + P, :], in_=out_sb[:])

    nc.finalize()
    return nc


_CACHE = {}


def run(inputs: dict, trace: bool = False, tmpdir: str | None = None):
    from concourse.bass_utils import run_bass_kernel_spmd

    in_maps, shapes, perm = build_host_inputs(inputs)
    key = repr(shapes)
    if key not in _CACHE:
        _CACHE.clear()
        _CACHE[key] = build_nc(shapes)
    nc = _CACHE[key]
    res = run_bass_kernel_spmd(
        nc, in_maps, core_ids=list(range(N_CORES)), trace=trace, tmpdir=tmpdir
    )
    out = np.zeros((B, E), np.float32)
    for c in range(N_CORES):
        out[perm[c]] = np.asarray(res.results[c]["out"], dtype=np.float32)
    return out, res


def kernel(**inputs) -> np.ndarray:
    out, _ = run(inputs, trace=False)
    return out



# revision 5
# speedup vs baseline: 1.1852x; 1.1852x over previous
"""AEM attention-pooling kernel for 8 Trainium2 NeuronCores.

Strategy: data-parallel over batch (512 rows/core), embedding tables
replicated, zero collectives.

Gather engine: dma_gather (int16 indices) with the tables split into
32768-row windows. Host-side slot assignment places each batch row's
items in its own SBUF partition: for window w, the gather index list is
arranged so list position k = c*128 + p holds the c-th in-window item of
the batch row assigned to partition p (dma_gather writes position k to
partition k%128, free chunk k//128). Slots beyond a row's in-window
count point at a pad row and are masked out of the softmax. The word
table gets an explicit zero row per window so word pads vanish from the
mean without masking.

Math per row b (red_w folded into the attention vector):
    qm = mean_j word_emb[qw[b,j]];  q = tanh(qp_w @ qm + qp_b)
    v  = sum_h red_w[h] * tanh(aq_w_h @ q + aq_b_h)
    s_c = bought[b,c,:] . v;  w = softmax_masked(s);  user = sum w_c bought_c
    out = 0.5*(q + user)

Engines: 6 consolidated dma_gathers on GPSIMD (one per table window,
spread over 4 SWDGE queues, words first so the MLP overlaps item
gathers); scores/weighted-rows as blockwise broadcast-multiplies +
reduces on VectorE; word-mean and attention-weighted sums as N=512
batched identity-matmul PSUM accumulations on TensorE; tanh/exp on
ScalarE. Indices are int16 (dma_gather requirement), hence the
32768-row table windows.
"""

import numpy as np

try:
    import ml_dtypes

    _BF16 = ml_dtypes.bfloat16
except ImportError:  # pragma: no cover
    _BF16 = None

B, I, QW = 4096, 50, 20
WORD_NUM, ITEM_NUM, E, H = 50000, 100000, 128, 8
N_CORES = 8
B_CORE = B // N_CORES
P = 128
WIN = 32768
ITEM_WINS = 4   # ceil(100000 / 32768)
WORD_WINS = 2   # padded word table: window bases 0 and 32768

ITEM_WIN_BASES = [0, WIN, 2 * WIN, 3 * WIN]
WORD_WIN_BASES = [0, WIN]
WORD_V_PADDED = WORD_NUM + 2  # zero row at 0 and at 32768


def plan_layout(ubi, qw_arr):
    """Host planning: per-core batch-row assignment + shared chunk caps.

    Returns perm[core] ([512] original batch row for slot tile*128+p) and
    per-tile-slot window chunk caps shared across cores (SPMD needs one
    graph shape for all cores).
    """
    # GLOBAL sort of all batch rows by item window profile, then deal rows so
    # that tile-slot t of every core draws from the same profile segment --
    # the shared (max-over-cores) caps stay tight.
    ic_all = np.stack(
        [((ubi >= w * WIN) & (ubi < (w + 1) * WIN)).sum(1) for w in range(ITEM_WINS)],
        axis=1,
    )
    order_all = np.lexsort((ic_all[:, 2], ic_all[:, 1], ic_all[:, 0]))
    n_tiles_ = B_CORE // P
    perm = np.zeros((N_CORES, B_CORE), np.int64)
    item_cnt = np.zeros((N_CORES, B_CORE, ITEM_WINS), np.int64)
    word_cnt = np.zeros((N_CORES, B_CORE, WORD_WINS), np.int64)
    for t in range(n_tiles_):
        seg = order_all[t * P * N_CORES : (t + 1) * P * N_CORES]
        for c in range(N_CORES):
            rows = seg[c * P : (c + 1) * P]
            perm[c, t * P : (t + 1) * P] = rows
            item_cnt[c, t * P : (t + 1) * P] = ic_all[rows]
            wc0 = (qw_arr[rows] < (WIN - 1)).sum(1)
            word_cnt[c, t * P : (t + 1) * P] = np.stack([wc0, QW - wc0], axis=1)
    n_tiles = B_CORE // P
    item_caps = np.zeros((n_tiles, ITEM_WINS), np.int64)
    word_caps = np.zeros((n_tiles, WORD_WINS), np.int64)
    for t in range(n_tiles):
        item_caps[t] = item_cnt[:, t * P : (t + 1) * P].max(axis=(0, 1))
        word_caps[t] = word_cnt[:, t * P : (t + 1) * P].max(axis=(0, 1))
    return perm, item_caps, word_caps


def _wrap16(lst):
    """dma_gather index wrapping: position g -> (partition g%16, slot g//16),
    replicated to all 8 16-partition groups."""
    n = lst.shape[0]
    assert n % 16 == 0
    w = lst.reshape(n // 16, 16).T.astype(np.int16)  # [16, n/16]
    return np.tile(w, (8, 1))  # [128, n/16]


def build_host_inputs(inputs):
    """All host prep: planning, index lists, masks, tables, weights."""
    ubi = np.asarray(inputs["user_bought_items"]).astype(np.int64)
    qw_arr = np.asarray(inputs["query_words"]).astype(np.int64)
    masks_in = np.asarray(inputs["user_bought_masks"], dtype=np.float32).reshape(B, I)
    word_emb = np.asarray(inputs["word_emb"], dtype=np.float32)
    item_emb = np.asarray(inputs["item_emb"], dtype=np.float32)
    qp_w = np.asarray(inputs["qp_w"], dtype=np.float32)
    qp_b = np.asarray(inputs["qp_b"], dtype=np.float32)
    aq_w = np.asarray(inputs["aq_w"], dtype=np.float32)
    aq_b = np.asarray(inputs["aq_b"], dtype=np.float32)
    red_w = np.asarray(inputs["red_w"], dtype=np.float32)

    perm, item_caps, word_caps = plan_layout(ubi, qw_arr)
    n_tiles = B_CORE // P
    CH = [int(item_caps[t].sum()) for t in range(n_tiles)]
    CW = [int(word_caps[t].sum()) for t in range(n_tiles)]

    # padded word table: zero rows at 0 and 32768
    # remap: old r < 32767 -> r+1 (window 0); old r >= 32767 -> r+2 (window 1)
    word_tab = np.zeros((WORD_V_PADDED, E), np.float32)
    word_tab[1:WIN] = word_emb[: WIN - 1]
    word_tab[WIN + 1 :] = word_emb[WIN - 1 :]
    word_tab_b16 = np.ascontiguousarray(word_tab.astype(_BF16))
    item_tab_b16 = np.ascontiguousarray(item_emb.astype(_BF16))

    # index lists grouped per WINDOW across all tiles (one gather per window);
    # within a window, tile t's block occupies chunks [sum_t'<t caps[t'][w], ...)
    item_idx_packed = []
    word_idx_packed = []
    mask_packed = []
    for c in range(N_CORES):
        ilists = [[] for _ in range(ITEM_WINS)]   # per window: chunk lists per tile
        wlists = [[] for _ in range(WORD_WINS)]
        mparts = []
        for t in range(n_tiles):
            rows = perm[c, t * P : (t + 1) * P]
            sub_ubi = ubi[rows]
            sub_qw = qw_arr[rows]
            sub_mask = masks_in[rows]
            mtile = np.zeros((P, CH[t]), np.float32)
            off = 0
            for w in range(ITEM_WINS):
                cap = int(item_caps[t, w])
                lst = np.zeros(cap * P, np.int64)  # pad 0 = window base row
                for p in range(P):
                    iw = np.where((sub_ubi[p] >= w * WIN) & (sub_ubi[p] < (w + 1) * WIN))[0]
                    loc = sub_ubi[p, iw] - w * WIN
                    for c2, (li, ii) in enumerate(zip(loc, iw)):
                        lst[c2 * P + p] = li
                        mtile[p, off + c2] = sub_mask[p, ii]
                ilists[w].append(lst)
                off += cap
            for w in range(WORD_WINS):
                cap = int(word_caps[t, w])
                lst = np.zeros(cap * P, np.int64)  # pad 0 = zero row
                for p in range(P):
                    if w == 0:
                        jw = np.where(sub_qw[p] < (WIN - 1))[0]
                        loc = sub_qw[p, jw] + 1
                    else:
                        jw = np.where(sub_qw[p] >= (WIN - 1))[0]
                        loc = sub_qw[p, jw] + 2 - WIN
                    for c2, li in enumerate(loc):
                        lst[c2 * P + p] = li
                wlists[w].append(lst)
            mparts.append(mtile)
        iparts = [_wrap16(np.concatenate(ilists[w])) for w in range(ITEM_WINS)]
        wparts = [_wrap16(np.concatenate(wlists[w])) for w in range(WORD_WINS)]
        item_idx_packed.append(np.concatenate(iparts, axis=1))
        word_idx_packed.append(np.concatenate(wparts, axis=1))
        mask_packed.append(np.concatenate(mparts, axis=1))

    qp_wT = np.ascontiguousarray(qp_w.T.astype(_BF16))
    aq_wT = np.ascontiguousarray(aq_w.T.astype(_BF16))
    qp_b_c = np.ascontiguousarray(qp_b.reshape(E, 1))
    aq_bT = np.ascontiguousarray(aq_b.reshape(H, E).T)
    red_w_r = np.ascontiguousarray(red_w.reshape(1, H))

    in_maps = []
    for c in range(N_CORES):
        in_maps.append(
            {
                "item_idx": item_idx_packed[c],
                "word_idx": word_idx_packed[c],
                "maskp": mask_packed[c],
                "word_tab": word_tab_b16,
                "item_tab": item_tab_b16,
                "qp_wT": qp_wT,
                "qp_b": qp_b_c,
                "aq_wT": aq_wT,
                "aq_bT": aq_bT,
                "red_w": red_w_r,
            }
        )
    shapes = {
        "item_caps": [[int(x) for x in row] for row in item_caps],
        "word_caps": [[int(x) for x in row] for row in word_caps],
        "CH": CH,
        "CW": CW,
        "item_idx_w": int(item_idx_packed[0].shape[1]),
        "word_idx_w": int(word_idx_packed[0].shape[1]),
        "mask_w": int(mask_packed[0].shape[1]),
    }
    return in_maps, shapes, perm


def build_nc(shapes):
    import concourse.bacc as bacc
    import concourse.mybir as mybir
    from concourse.masks import make_identity
    from concourse.tile import TileContext

    f32 = mybir.dt.float32
    bf16 = mybir.dt.bfloat16
    i16 = mybir.dt.int16
    AF = mybir.ActivationFunctionType
    OP = mybir.AluOpType

    item_caps = shapes["item_caps"]
    word_caps = shapes["word_caps"]
    CH = shapes["CH"]
    CW = shapes["CW"]
    n_tiles = len(CH)
    ITOT = [sum(item_caps[t][w] for t in range(n_tiles)) for w in range(ITEM_WINS)]
    WTOT = [sum(word_caps[t][w] for t in range(n_tiles)) for w in range(WORD_WINS)]
    SCH = sum(ITOT)
    SCW = sum(WTOT)
    # global chunk offset of tile t's window-w block in bought_all / wg_all
    IW_START = [sum(ITOT[:w]) for w in range(ITEM_WINS)]
    WW_START = [sum(WTOT[:w]) for w in range(WORD_WINS)]

    def gcol_item(t, w):
        return IW_START[w] + sum(item_caps[tt][w] for tt in range(t))

    def gcol_word(t, w):
        return WW_START[w] + sum(word_caps[tt][w] for tt in range(t))

    nc = bacc.Bacc("TRN2", num_swdge_queues=4)

    item_idx = nc.dram_tensor("item_idx", [P, shapes["item_idx_w"]], i16, kind="ExternalInput")
    word_idx = nc.dram_tensor("word_idx", [P, shapes["word_idx_w"]], i16, kind="ExternalInput")
    maskp = nc.dram_tensor("maskp", [P, shapes["mask_w"]], f32, kind="ExternalInput")
    word_tab = nc.dram_tensor("word_tab", [WORD_V_PADDED, E], bf16, kind="ExternalInput")
    item_tab = nc.dram_tensor("item_tab", [ITEM_NUM, E], bf16, kind="ExternalInput")
    qp_wT = nc.dram_tensor("qp_wT", [E, E], bf16, kind="ExternalInput")
    qp_b = nc.dram_tensor("qp_b", [E, 1], f32, kind="ExternalInput")
    aq_wT = nc.dram_tensor("aq_wT", [E, E * H], bf16, kind="ExternalInput")
    aq_bT = nc.dram_tensor("aq_bT", [E, H], f32, kind="ExternalInput")
    red_w = nc.dram_tensor("red_w", [1, H], f32, kind="ExternalInput")
    out = nc.dram_tensor("out", [B_CORE, E], f32, kind="ExternalOutput")

    with TileContext(nc) as tc:
        with (
            tc.tile_pool(name="const", bufs=1) as cpool,
            tc.tile_pool(name="big", bufs=1) as big,
            tc.tile_pool(name="mid", bufs=2) as mid,
            tc.tile_pool(name="small", bufs=3) as small,
            tc.tile_pool(name="psum_acc", bufs=1, space="PSUM") as pacc,
            tc.tile_pool(name="psum_tr", bufs=2, space="PSUM") as ptr,
        ):
            ident = cpool.tile([P, P], bf16, tag="ident")
            make_identity(nc, ident[:])
            ident_f = cpool.tile([P, P], f32, tag="ident_f")
            make_identity(nc, ident_f[:])

            qp_wT_sb = cpool.tile([E, E], bf16, tag="qp_wT_sb")
            nc.sync.dma_start(out=qp_wT_sb[:], in_=qp_wT[:, :])
            aq_wT_sb = cpool.tile([E, E * H], bf16, tag="aq_wT_sb")
            nc.sync.dma_start(out=aq_wT_sb[:], in_=aq_wT[:, :])
            qp_b_sb = cpool.tile([E, 1], f32, tag="qp_b_sb")
            nc.sync.dma_start(out=qp_b_sb[:], in_=qp_b[:, :])
            aq_bT_sb = cpool.tile([E, H], f32, tag="aq_bT_sb")
            nc.sync.dma_start(out=aq_bT_sb[:], in_=aq_bT[:, :])
            red_w_sb = cpool.tile([1, H], f32, tag="red_w_sb")
            nc.sync.dma_start(out=red_w_sb[:], in_=red_w[:, :])

            ones_col = cpool.tile([1, P], f32, tag="ones_col")
            nc.vector.memset(ones_col[:], 1.0)
            rw_psum = ptr.tile([P, H], f32, tag="mm", space="PSUM")
            nc.tensor.matmul(out=rw_psum[:], lhsT=ones_col[:], rhs=red_w_sb[:], start=True, stop=True)
            rw_bcast = cpool.tile([P, H], f32, tag="rw_bcast")
            nc.scalar.copy(out=rw_bcast[:], in_=rw_psum[:])

            scaledI = cpool.tile([P, H * P], bf16, tag="scaledI")
            for h in range(H):
                nc.vector.tensor_scalar_mul(
                    out=scaledI[:, h * P : (h + 1) * P], in0=ident[:],
                    scalar1=rw_bcast[:, h : h + 1],
                )

            # ---- index tiles + per-(tile,window) gathers on all 4 queues ----
            # Separate SBUF tiles per block give the scheduler fine-grained
            # deps (tile t's compute starts when ITS blocks land), and
            # spreading blocks over all 4 SWDGE queues runs 4 desc-gen
            # streams concurrently (measured ~2.2x vs consolidated).
            iidx_sb = cpool.tile([P, shapes["item_idx_w"]], i16, tag="iidx_sb")
            nc.sync.dma_start(out=iidx_sb[:], in_=item_idx[:, :])
            widx_sb = cpool.tile([P, shapes["word_idx_w"]], i16, tag="widx_sb")
            nc.sync.dma_start(out=widx_sb[:], in_=word_idx[:, :])

            wg_tw = {}
            bought_tw = {}
            for t in range(n_tiles):
                for w in range(WORD_WINS):
                    cap = int(word_caps[t][w])
                    if cap:
                        wg_tw[(t, w)] = cpool.tile(
                            [P, cap, E], bf16, name=f"wg_{t}_{w}", tag=f"wg_{t}_{w}")
                for w in range(ITEM_WINS):
                    cap = int(item_caps[t][w])
                    if cap:
                        bought_tw[(t, w)] = cpool.tile(
                            [P, cap, E], bf16, name=f"bt_{t}_{w}", tag=f"bt_{t}_{w}")

            qn = 0
            # words first, tile-major (tile 0's MLP inputs land first)
            for t in range(n_tiles):
                for w in range(WORD_WINS):
                    cap = int(word_caps[t][w])
                    if cap == 0:
                        continue
                    n = cap * P
                    g0 = gcol_word(t, w)
                    src = word_tab[WORD_WIN_BASES[w] :, :] if w else word_tab[:, :]
                    nc.gpsimd.dma_gather(
                        out_ap=wg_tw[(t, w)][:, :, :], in_ap=src,
                        idxs_ap=widx_sb[:, g0 * 8 : (g0 + cap) * 8],
                        num_idxs=n, num_idxs_reg=n, elem_size=E,
                        single_packet=False, queue_num=qn % 4,
                    )
                    qn += 1
            # items tile-major: tile t's 4 window blocks drain before t+1's
            for t in range(n_tiles):
                for w in range(ITEM_WINS):
                    cap = int(item_caps[t][w])
                    if cap == 0:
                        continue
                    n = cap * P
                    g0 = gcol_item(t, w)
                    src = item_tab[ITEM_WIN_BASES[w] :, :] if w else item_tab[:, :]
                    nc.gpsimd.dma_gather(
                        out_ap=bought_tw[(t, w)][:, :, :], in_ap=src,
                        idxs_ap=iidx_sb[:, g0 * 8 : (g0 + cap) * 8],
                        num_idxs=n, num_idxs_reg=n, elem_size=E,
                        single_packet=False, queue_num=qn % 4,
                    )
                    qn += 1

            m_off = 0
            for t in range(n_tiles):
                ch = CH[t]
                cw = CW[t]
                b0 = t * P

                mask_sb = small.tile([P, ch], f32, tag="mask_sb")
                nc.sync.dma_start(out=mask_sb[:], in_=maskp[:, m_off : m_off + ch])
                m_off += ch

                # ---- word mean via N=512 batched identity-matmuls ----
                qm4_psum = pacc.tile([P, 4 * E], f32, tag="qm4", space="PSUM")
                ngroups = []
                for w in range(WORD_WINS):
                    cap = int(word_caps[t][w])
                    c = 0
                    while c < cap:
                        r = min(4, cap - c)
                        ngroups.append((w, c, r))
                        c += r
                for gi, (w, c0, r) in enumerate(ngroups):
                    nc.tensor.matmul(
                        out=qm4_psum[:, : r * E],
                        lhsT=ident[:],
                        rhs=wg_tw[(t, w)][:, c0 : c0 + r, :].rearrange("p c e -> p (c e)"),
                        start=(gi == 0), stop=(gi == len(ngroups) - 1),
                    )
                qm_c1 = small.tile([P, E], f32, tag="qm_c1")
                nc.scalar.copy(out=qm_c1[:], in_=qm4_psum[:, E : 2 * E])
                qm_c3 = small.tile([P, E], f32, tag="qm_c3")
                nc.scalar.copy(out=qm_c3[:], in_=qm4_psum[:, 3 * E : 4 * E])
                qm_a = small.tile([P, E], f32, tag="qm_a")
                nc.vector.tensor_tensor(out=qm_a[:], in0=qm4_psum[:, 0:E], in1=qm_c1[:], op=OP.add)
                qm_b = small.tile([P, E], f32, tag="qm_b")
                nc.vector.tensor_tensor(out=qm_b[:], in0=qm4_psum[:, 2 * E : 3 * E], in1=qm_c3[:], op=OP.add)
                qm_sb = small.tile([P, E], f32, tag="qm_sb")
                nc.vector.tensor_tensor(out=qm_sb[:], in0=qm_a[:], in1=qm_b[:], op=OP.add)

                qmT_psum = ptr.tile([E, P], f32, tag="mmt", space="PSUM")
                nc.tensor.transpose(out=qmT_psum[:], in_=qm_sb[:], identity=ident_f[:])
                qmT_sb = small.tile([E, P], bf16, tag="qmT_sb")
                nc.scalar.mul(out=qmT_sb[:], in_=qmT_psum[:], mul=1.0 / QW)

                mm1_psum = ptr.tile([E, P], f32, tag="mm", space="PSUM")
                nc.tensor.matmul(out=mm1_psum[:], lhsT=qp_wT_sb[:], rhs=qmT_sb[:], start=True, stop=True)
                qT_f = small.tile([E, P], f32, tag="qT_f")
                nc.scalar.activation(out=qT_f[:], in_=mm1_psum[:], func=AF.Tanh, bias=qp_b_sb[:, 0:1])
                qT_sb = small.tile([E, P], bf16, tag="qT_sb")
                nc.vector.tensor_copy(out=qT_sb[:], in_=qT_f[:])

                q_psum = ptr.tile([P, E], f32, tag="mmt", space="PSUM")
                nc.tensor.transpose(out=q_psum[:], in_=qT_f[:], identity=ident_f[:])
                qhalf_sb = small.tile([P, E], f32, tag="qhalf_sb")
                nc.scalar.mul(out=qhalf_sb[:], in_=q_psum[:], mul=0.5)

                vT_psum = pacc.tile([E, P], f32, tag="vT", space="PSUM")
                for h in range(H):
                    mm2_psum = ptr.tile([E, P], f32, tag="mm", space="PSUM")
                    nc.tensor.matmul(
                        out=mm2_psum[:], lhsT=aq_wT_sb[:, h * E : (h + 1) * E],
                        rhs=qT_sb[:], start=True, stop=True,
                    )
                    t_h = small.tile([E, P], bf16, tag="t_h")
                    nc.scalar.activation(
                        out=t_h[:], in_=mm2_psum[:], func=AF.Tanh, bias=aq_bT_sb[:, h : h + 1]
                    )
                    nc.tensor.matmul(
                        out=vT_psum[:], lhsT=scaledI[:, h * P : (h + 1) * P],
                        rhs=t_h[:], start=(h == 0), stop=(h == H - 1),
                    )
                vT_sb = small.tile([E, P], f32, tag="vT_sb")
                nc.scalar.copy(out=vT_sb[:], in_=vT_psum[:])
                v_psum = ptr.tile([P, E], f32, tag="mmt", space="PSUM")
                nc.tensor.transpose(out=v_psum[:], in_=vT_sb[:], identity=ident_f[:])
                v_sb = small.tile([P, E], bf16, tag="v_sb")
                nc.scalar.copy(out=v_sb[:], in_=v_psum[:])

                iblocks = []
                for w in range(ITEM_WINS):
                    cap = int(item_caps[t][w])
                    if cap:
                        iblocks.append((w, cap))

                # ---- scores: blockwise bought * v-broadcast, reduce over E ----
                scores = small.tile([P, ch], f32, tag="scores")
                loff = 0
                for w, cap in iblocks:
                    prodS = mid.tile([P, cap, E], bf16, tag="prodS")
                    nc.vector.tensor_tensor(
                        out=prodS[:, :, :],
                        in0=bought_tw[(t, w)][:, :, :],
                        in1=v_sb[:, None, :].to_broadcast([P, cap, E]),
                        op=OP.mult,
                    )
                    nc.vector.tensor_reduce(
                        out=scores[:, loff : loff + cap], in_=prodS[:, :, :],
                        axis=mybir.AxisListType.X, op=OP.add,
                    )
                    loff += cap

                negmax = small.tile([P, 1], f32, tag="negmax")
                nc.vector.reduce_max(out=negmax[:], in_=scores[:], axis=mybir.AxisListType.X, negate=True)
                att = small.tile([P, ch], f32, tag="att")
                nc.scalar.activation(out=att[:], in_=scores[:], func=AF.Exp, bias=negmax[:, 0:1])
                attm = small.tile([P, ch], f32, tag="attm")
                nc.vector.tensor_tensor(out=attm[:], in0=att[:], in1=mask_sb[:], op=OP.mult)
                denom = small.tile([P, 1], f32, tag="denom")
                nc.vector.reduce_sum(out=denom[:], in_=attm[:], axis=mybir.AxisListType.X)
                lt01 = small.tile([P, 1], f32, tag="lt01")
                nc.vector.tensor_scalar(out=lt01[:], in0=denom[:], scalar1=1e-7, scalar2=None, op0=OP.is_lt)
                denom2 = small.tile([P, 1], f32, tag="denom2")
                nc.vector.tensor_tensor(out=denom2[:], in0=denom[:], in1=lt01[:], op=OP.add)
                nc.vector.tensor_scalar_mul(out=denom2[:], in0=denom2[:], scalar1=2.0)
                rcp = small.tile([P, 1], f32, tag="rcp")
                nc.vector.reciprocal(out=rcp[:], in_=denom2[:])

                # attm as bf16 for the big product
                attm_b = small.tile([P, ch], bf16, tag="attm_b")
                nc.vector.tensor_copy(out=attm_b[:], in_=attm[:])

                # ---- user: blockwise bought * attm-broadcast + batched matmuls ----
                prodU = big.tile([P, ch, E], bf16, tag="prodU")
                loff = 0
                for w, cap in iblocks:
                    nc.vector.tensor_tensor(
                        out=prodU[:, loff : loff + cap, :],
                        in0=bought_tw[(t, w)][:, :, :],
                        in1=attm_b[:, loff : loff + cap, None].to_broadcast([P, cap, E]),
                        op=OP.mult,
                    )
                    loff += cap
                u4_psum = pacc.tile([P, 4 * E], f32, tag="u4", space="PSUM")
                ngroups = (ch + 3) // 4
                for g in range(ngroups):
                    c0 = g * 4
                    r = min(4, ch - c0)
                    nc.tensor.matmul(
                        out=u4_psum[:, : r * E],
                        lhsT=ident[:],
                        rhs=prodU[:, c0 : c0 + r, :].rearrange("p c e -> p (c e)"),
                        start=(g == 0), stop=(g == ngroups - 1),
                    )
                u_c1 = small.tile([P, E], f32, tag="u_c1")
                nc.scalar.copy(out=u_c1[:], in_=u4_psum[:, E : 2 * E])
                u_c3 = small.tile([P, E], f32, tag="u_c3")
                nc.scalar.copy(out=u_c3[:], in_=u4_psum[:, 3 * E : 4 * E])
                u_a = small.tile([P, E], f32, tag="u_a")
                nc.vector.tensor_tensor(out=u_a[:], in0=u4_psum[:, 0:E], in1=u_c1[:], op=OP.add)
                u_b = small.tile([P, E], f32, tag="u_b")
                nc.vector.tensor_tensor(out=u_b[:], in0=u4_psum[:, 2 * E : 3 * E], in1=u_c3[:], op=OP.add)
                user_sb = small.tile([P, E], f32, tag="user_sb")
                nc.vector.tensor_tensor(out=user_sb[:], in0=u_a[:], in1=u_b[:], op=OP.add)

                out_sb = small.tile([P, E], f32, tag="out_sb")
                nc.vector.scalar_tensor_tensor(
                    out=out_sb[:], in0=user_sb[:], scalar=rcp[:, 0:1],
                    in1=qhalf_sb[:], op0=OP.mult, op1=OP.add,
                )
                nc.sync.dma_start(out=out[b0 : b0 + P, :], in_=out_sb[:])

    nc.finalize()
    return nc


_CACHE = {}


def run(inputs: dict, trace: bool = False, tmpdir: str | None = None):
    from concourse.bass_utils import run_bass_kernel_spmd

    in_maps, shapes, perm = build_host_inputs(inputs)
    key = repr(shapes)
    if key not in _CACHE:
        _CACHE.clear()
        _CACHE[key] = build_nc(shapes)
    nc = _CACHE[key]
    res = run_bass_kernel_spmd(
        nc, in_maps, core_ids=list(range(N_CORES)), trace=trace, tmpdir=tmpdir
    )
    out = np.zeros((B, E), np.float32)
    for c in range(N_CORES):
        out[perm[c]] = np.asarray(res.results[c]["out"], dtype=np.float32)
    return out, res


def kernel(**inputs) -> np.ndarray:
    out, _ = run(inputs, trace=False)
    return out



# revision 6
# speedup vs baseline: 1.2480x; 1.0530x over previous
"""AEM attention-pooling kernel for 8 Trainium2 NeuronCores.

Strategy: data-parallel over batch (512 rows/core), embedding tables
replicated, zero collectives.

Gather engine: dma_gather (int16 indices) with the tables split into
32768-row windows. Host-side slot assignment places each batch row's
items in its own SBUF partition: for window w, the gather index list is
arranged so list position k = c*128 + p holds the c-th in-window item of
the batch row assigned to partition p (dma_gather writes position k to
partition k%128, free chunk k//128). Slots beyond a row's in-window
count point at a pad row and are masked out of the softmax. The word
table gets an explicit zero row per window so word pads vanish from the
mean without masking.

Math per row b (red_w folded into the attention vector):
    qm = mean_j word_emb[qw[b,j]];  q = tanh(qp_w @ qm + qp_b)
    v  = sum_h red_w[h] * tanh(aq_w_h @ q + aq_b_h)
    s_c = bought[b,c,:] . v;  w = softmax_masked(s);  user = sum w_c bought_c
    out = 0.5*(q + user)

Engines: 6 consolidated dma_gathers on GPSIMD (one per table window,
spread over 4 SWDGE queues, words first so the MLP overlaps item
gathers); scores/weighted-rows as blockwise broadcast-multiplies +
reduces on VectorE; word-mean and attention-weighted sums as N=512
batched identity-matmul PSUM accumulations on TensorE; tanh/exp on
ScalarE. Indices are int16 (dma_gather requirement), hence the
32768-row table windows.
"""

import numpy as np

try:
    import ml_dtypes

    _BF16 = ml_dtypes.bfloat16
except ImportError:  # pragma: no cover
    _BF16 = None

B, I, QW = 4096, 50, 20
WORD_NUM, ITEM_NUM, E, H = 50000, 100000, 128, 8
N_CORES = 8
B_CORE = B // N_CORES
P = 128
WIN = 32768
ITEM_WINS = 4   # ceil(100000 / 32768)
WORD_WINS = 2   # padded word table: window bases 0 and 32768

ITEM_WIN_BASES = [0, WIN, 2 * WIN, 3 * WIN]
WORD_WIN_BASES = [0, WIN]
WORD_V_PADDED = WORD_NUM + 2  # zero row at 0 and at 32768


def plan_layout(ubi, qw_arr):
    """Host planning: per-core batch-row assignment + shared chunk caps.

    Returns perm[core] ([512] original batch row for slot tile*128+p) and
    per-tile-slot window chunk caps shared across cores (SPMD needs one
    graph shape for all cores).
    """
    # GLOBAL sort of all batch rows by item window profile, then deal rows so
    # that tile-slot t of every core draws from the same profile segment --
    # the shared (max-over-cores) caps stay tight.
    ic_all = np.stack(
        [((ubi >= w * WIN) & (ubi < (w + 1) * WIN)).sum(1) for w in range(ITEM_WINS)],
        axis=1,
    )
    order_all = np.lexsort((ic_all[:, 2], ic_all[:, 1], ic_all[:, 0]))
    n_tiles_ = B_CORE // P
    perm = np.zeros((N_CORES, B_CORE), np.int64)
    item_cnt = np.zeros((N_CORES, B_CORE, ITEM_WINS), np.int64)
    word_cnt = np.zeros((N_CORES, B_CORE, WORD_WINS), np.int64)
    for t in range(n_tiles_):
        seg = order_all[t * P * N_CORES : (t + 1) * P * N_CORES]
        for c in range(N_CORES):
            rows = seg[c * P : (c + 1) * P]
            perm[c, t * P : (t + 1) * P] = rows
            item_cnt[c, t * P : (t + 1) * P] = ic_all[rows]
            wc0 = (qw_arr[rows] < (WIN - 1)).sum(1)
            word_cnt[c, t * P : (t + 1) * P] = np.stack([wc0, QW - wc0], axis=1)
    n_tiles = B_CORE // P
    item_caps = np.zeros((n_tiles, ITEM_WINS), np.int64)
    word_caps = np.zeros((n_tiles, WORD_WINS), np.int64)
    for t in range(n_tiles):
        item_caps[t] = item_cnt[:, t * P : (t + 1) * P].max(axis=(0, 1))
        word_caps[t] = word_cnt[:, t * P : (t + 1) * P].max(axis=(0, 1))
    return perm, item_caps, word_caps


def _wrap16(lst):
    """dma_gather index wrapping: position g -> (partition g%16, slot g//16),
    replicated to all 8 16-partition groups."""
    n = lst.shape[0]
    assert n % 16 == 0
    w = lst.reshape(n // 16, 16).T.astype(np.int16)  # [16, n/16]
    return np.tile(w, (8, 1))  # [128, n/16]


def build_host_inputs(inputs):
    """All host prep: planning, index lists, masks, tables, weights."""
    ubi = np.asarray(inputs["user_bought_items"]).astype(np.int64)
    qw_arr = np.asarray(inputs["query_words"]).astype(np.int64)
    masks_in = np.asarray(inputs["user_bought_masks"], dtype=np.float32).reshape(B, I)
    word_emb = np.asarray(inputs["word_emb"], dtype=np.float32)
    item_emb = np.asarray(inputs["item_emb"], dtype=np.float32)
    qp_w = np.asarray(inputs["qp_w"], dtype=np.float32)
    qp_b = np.asarray(inputs["qp_b"], dtype=np.float32)
    aq_w = np.asarray(inputs["aq_w"], dtype=np.float32)
    aq_b = np.asarray(inputs["aq_b"], dtype=np.float32)
    red_w = np.asarray(inputs["red_w"], dtype=np.float32)

    perm, item_caps, word_caps = plan_layout(ubi, qw_arr)
    n_tiles = B_CORE // P
    CH = [int(item_caps[t].sum()) for t in range(n_tiles)]
    CW = [int(word_caps[t].sum()) for t in range(n_tiles)]

    # padded word table: zero rows at 0 and 32768
    # remap: old r < 32767 -> r+1 (window 0); old r >= 32767 -> r+2 (window 1)
    word_tab = np.zeros((WORD_V_PADDED, E), np.float32)
    word_tab[1:WIN] = word_emb[: WIN - 1]
    word_tab[WIN + 1 :] = word_emb[WIN - 1 :]
    word_tab_b16 = np.ascontiguousarray(word_tab.astype(_BF16))
    item_tab_b16 = np.ascontiguousarray(item_emb.astype(_BF16))

    # index lists grouped per WINDOW across all tiles (one gather per window);
    # within a window, tile t's block occupies chunks [sum_t'<t caps[t'][w], ...)
    item_idx_packed = []
    word_idx_packed = []
    mask_packed = []
    for c in range(N_CORES):
        ilists = [[] for _ in range(ITEM_WINS)]   # per window: chunk lists per tile
        wlists = [[] for _ in range(WORD_WINS)]
        mparts = []
        for t in range(n_tiles):
            rows = perm[c, t * P : (t + 1) * P]
            sub_ubi = ubi[rows]
            sub_qw = qw_arr[rows]
            sub_mask = masks_in[rows]
            mtile = np.zeros((P, CH[t]), np.float32)
            off = 0
            for w in range(ITEM_WINS):
                cap = int(item_caps[t, w])
                lst = np.zeros(cap * P, np.int64)  # pad 0 = window base row
                for p in range(P):
                    iw = np.where((sub_ubi[p] >= w * WIN) & (sub_ubi[p] < (w + 1) * WIN))[0]
                    loc = sub_ubi[p, iw] - w * WIN
                    for c2, (li, ii) in enumerate(zip(loc, iw)):
                        lst[c2 * P + p] = li
                        mtile[p, off + c2] = sub_mask[p, ii]
                ilists[w].append(lst)
                off += cap
            for w in range(WORD_WINS):
                cap = int(word_caps[t, w])
                lst = np.zeros(cap * P, np.int64)  # pad 0 = zero row
                for p in range(P):
                    if w == 0:
                        jw = np.where(sub_qw[p] < (WIN - 1))[0]
                        loc = sub_qw[p, jw] + 1
                    else:
                        jw = np.where(sub_qw[p] >= (WIN - 1))[0]
                        loc = sub_qw[p, jw] + 2 - WIN
                    for c2, li in enumerate(loc):
                        lst[c2 * P + p] = li
                wlists[w].append(lst)
            mparts.append(mtile)
        iparts = [_wrap16(np.concatenate(ilists[w])) for w in range(ITEM_WINS)]
        wparts = [_wrap16(np.concatenate(wlists[w])) for w in range(WORD_WINS)]
        item_idx_packed.append(np.concatenate(iparts, axis=1))
        word_idx_packed.append(np.concatenate(wparts, axis=1))
        mask_packed.append(np.concatenate(mparts, axis=1))

    qp_wT = np.ascontiguousarray(qp_w.T.astype(_BF16))
    aq_wT = np.ascontiguousarray(aq_w.T.astype(_BF16))
    qp_b_c = np.ascontiguousarray(qp_b.reshape(E, 1))
    aq_bT = np.ascontiguousarray(aq_b.reshape(H, E).T)
    red_w_r = np.ascontiguousarray(red_w.reshape(1, H))

    in_maps = []
    for c in range(N_CORES):
        in_maps.append(
            {
                "item_idx": item_idx_packed[c],
                "word_idx": word_idx_packed[c],
                "maskp": mask_packed[c],
                "word_tab": word_tab_b16,
                "item_tab": item_tab_b16,
                "qp_wT": qp_wT,
                "qp_b": qp_b_c,
                "aq_wT": aq_wT,
                "aq_bT": aq_bT,
                "red_w": red_w_r,
            }
        )
    shapes = {
        "item_caps": [[int(x) for x in row] for row in item_caps],
        "word_caps": [[int(x) for x in row] for row in word_caps],
        "CH": CH,
        "CW": CW,
        "item_idx_w": int(item_idx_packed[0].shape[1]),
        "word_idx_w": int(word_idx_packed[0].shape[1]),
        "mask_w": int(mask_packed[0].shape[1]),
    }
    return in_maps, shapes, perm


def build_nc(shapes):
    import concourse.bacc as bacc
    import concourse.mybir as mybir
    from concourse.masks import make_identity
    from concourse.tile import TileContext

    f32 = mybir.dt.float32
    bf16 = mybir.dt.bfloat16
    i16 = mybir.dt.int16
    AF = mybir.ActivationFunctionType
    OP = mybir.AluOpType

    item_caps = shapes["item_caps"]
    word_caps = shapes["word_caps"]
    CH = shapes["CH"]
    CW = shapes["CW"]
    n_tiles = len(CH)
    ITOT = [sum(item_caps[t][w] for t in range(n_tiles)) for w in range(ITEM_WINS)]
    WTOT = [sum(word_caps[t][w] for t in range(n_tiles)) for w in range(WORD_WINS)]
    SCH = sum(ITOT)
    SCW = sum(WTOT)
    # global chunk offset of tile t's window-w block in bought_all / wg_all
    IW_START = [sum(ITOT[:w]) for w in range(ITEM_WINS)]
    WW_START = [sum(WTOT[:w]) for w in range(WORD_WINS)]

    def gcol_item(t, w):
        return IW_START[w] + sum(item_caps[tt][w] for tt in range(t))

    def gcol_word(t, w):
        return WW_START[w] + sum(word_caps[tt][w] for tt in range(t))

    nc = bacc.Bacc("TRN2", num_swdge_queues=4)

    item_idx = nc.dram_tensor("item_idx", [P, shapes["item_idx_w"]], i16, kind="ExternalInput")
    word_idx = nc.dram_tensor("word_idx", [P, shapes["word_idx_w"]], i16, kind="ExternalInput")
    maskp = nc.dram_tensor("maskp", [P, shapes["mask_w"]], f32, kind="ExternalInput")
    word_tab = nc.dram_tensor("word_tab", [WORD_V_PADDED, E], bf16, kind="ExternalInput")
    item_tab = nc.dram_tensor("item_tab", [ITEM_NUM, E], bf16, kind="ExternalInput")
    qp_wT = nc.dram_tensor("qp_wT", [E, E], bf16, kind="ExternalInput")
    qp_b = nc.dram_tensor("qp_b", [E, 1], f32, kind="ExternalInput")
    aq_wT = nc.dram_tensor("aq_wT", [E, E * H], bf16, kind="ExternalInput")
    aq_bT = nc.dram_tensor("aq_bT", [E, H], f32, kind="ExternalInput")
    red_w = nc.dram_tensor("red_w", [1, H], f32, kind="ExternalInput")
    out = nc.dram_tensor("out", [B_CORE, E], f32, kind="ExternalOutput")

    with TileContext(nc) as tc:
        with (
            tc.tile_pool(name="const", bufs=1) as cpool,
            tc.tile_pool(name="big", bufs=1) as big,
            tc.tile_pool(name="mid", bufs=2) as mid,
            tc.tile_pool(name="small", bufs=3) as small,
            tc.tile_pool(name="psum_acc", bufs=1, space="PSUM") as pacc,
            tc.tile_pool(name="psum_tr", bufs=2, space="PSUM") as ptr,
        ):
            ident = cpool.tile([P, P], bf16, tag="ident")
            make_identity(nc, ident[:])
            ident_f = cpool.tile([P, P], f32, tag="ident_f")
            make_identity(nc, ident_f[:])

            qp_wT_sb = cpool.tile([E, E], bf16, tag="qp_wT_sb")
            nc.sync.dma_start(out=qp_wT_sb[:], in_=qp_wT[:, :])
            aq_wT_sb = cpool.tile([E, E * H], bf16, tag="aq_wT_sb")
            nc.sync.dma_start(out=aq_wT_sb[:], in_=aq_wT[:, :])
            qp_b_sb = cpool.tile([E, 1], f32, tag="qp_b_sb")
            nc.sync.dma_start(out=qp_b_sb[:], in_=qp_b[:, :])
            aq_bT_sb = cpool.tile([E, H], f32, tag="aq_bT_sb")
            nc.sync.dma_start(out=aq_bT_sb[:], in_=aq_bT[:, :])
            red_w_sb = cpool.tile([1, H], f32, tag="red_w_sb")
            nc.sync.dma_start(out=red_w_sb[:], in_=red_w[:, :])

            ones_col = cpool.tile([1, P], f32, tag="ones_col")
            nc.vector.memset(ones_col[:], 1.0)
            rw_psum = ptr.tile([P, H], f32, tag="mm", space="PSUM")
            nc.tensor.matmul(out=rw_psum[:], lhsT=ones_col[:], rhs=red_w_sb[:], start=True, stop=True)
            rw_bcast = cpool.tile([P, H], f32, tag="rw_bcast")
            nc.scalar.copy(out=rw_bcast[:], in_=rw_psum[:])

            scaledI = cpool.tile([P, H * P], bf16, tag="scaledI")
            for h in range(H):
                nc.vector.tensor_scalar_mul(
                    out=scaledI[:, h * P : (h + 1) * P], in0=ident[:],
                    scalar1=rw_bcast[:, h : h + 1],
                )

            # ---- index tiles + per-(tile,window) gathers on all 4 queues ----
            # Separate SBUF tiles per block give the scheduler fine-grained
            # deps (tile t's compute starts when ITS blocks land), and
            # spreading blocks over all 4 SWDGE queues runs 4 desc-gen
            # streams concurrently (measured ~2.2x vs consolidated).
            iidx_sb = cpool.tile([P, shapes["item_idx_w"]], i16, tag="iidx_sb")
            nc.sync.dma_start(out=iidx_sb[:], in_=item_idx[:, :])
            widx_sb = cpool.tile([P, shapes["word_idx_w"]], i16, tag="widx_sb")
            nc.sync.dma_start(out=widx_sb[:], in_=word_idx[:, :])

            wg_tw = {}
            bought_tw = {}
            for t in range(n_tiles):
                for w in range(WORD_WINS):
                    cap = int(word_caps[t][w])
                    if cap:
                        wg_tw[(t, w)] = cpool.tile(
                            [P, cap, E], bf16, name=f"wg_{t}_{w}", tag=f"wg_{t}_{w}")
                for w in range(ITEM_WINS):
                    cap = int(item_caps[t][w])
                    if cap:
                        bought_tw[(t, w)] = cpool.tile(
                            [P, cap, E], bf16, name=f"bt_{t}_{w}", tag=f"bt_{t}_{w}")

            qn = 0
            # words first, tile-major (tile 0's MLP inputs land first)
            for t in range(n_tiles):
                for w in range(WORD_WINS):
                    cap = int(word_caps[t][w])
                    if cap == 0:
                        continue
                    n = cap * P
                    g0 = gcol_word(t, w)
                    src = word_tab[WORD_WIN_BASES[w] :, :] if w else word_tab[:, :]
                    nc.gpsimd.dma_gather(
                        out_ap=wg_tw[(t, w)][:, :, :], in_ap=src,
                        idxs_ap=widx_sb[:, g0 * 8 : (g0 + cap) * 8],
                        num_idxs=n, num_idxs_reg=n, elem_size=E,
                        single_packet=False, queue_num=qn % 4,
                    )
                    qn += 1
            # items tile-major: tile t's window blocks drain before t+1's.
            # Each block is split in half: the Pool exec queue is 4 deep and
            # each gather holds its slot until its DMAs drain, so smaller
            # gathers pipeline across tile boundaries with smaller bubbles.
            for t in range(n_tiles):
                for w in range(ITEM_WINS):
                    cap = int(item_caps[t][w])
                    if cap == 0:
                        continue
                    g0 = gcol_item(t, w)
                    src = item_tab[ITEM_WIN_BASES[w] :, :] if w else item_tab[:, :]
                    halves = [(0, cap // 2), (cap // 2, cap - cap // 2)]
                    for hc0, hcap in halves:
                        if hcap == 0:
                            continue
                        n = hcap * P
                        nc.gpsimd.dma_gather(
                            out_ap=bought_tw[(t, w)][:, hc0 : hc0 + hcap, :],
                            in_ap=src,
                            idxs_ap=iidx_sb[:, (g0 + hc0) * 8 : (g0 + hc0 + hcap) * 8],
                            num_idxs=n, num_idxs_reg=n, elem_size=E,
                            single_packet=False, queue_num=qn % 4,
                        )
                        qn += 1

            m_off = 0
            for t in range(n_tiles):
                ch = CH[t]
                cw = CW[t]
                b0 = t * P

                mask_sb = small.tile([P, ch], f32, tag="mask_sb")
                nc.sync.dma_start(out=mask_sb[:], in_=maskp[:, m_off : m_off + ch])
                m_off += ch

                # ---- word mean via N=512 batched identity-matmuls ----
                qm4_psum = pacc.tile([P, 4 * E], f32, tag="qm4", space="PSUM")
                ngroups = []
                for w in range(WORD_WINS):
                    cap = int(word_caps[t][w])
                    c = 0
                    while c < cap:
                        r = min(4, cap - c)
                        ngroups.append((w, c, r))
                        c += r
                for gi, (w, c0, r) in enumerate(ngroups):
                    nc.tensor.matmul(
                        out=qm4_psum[:, : r * E],
                        lhsT=ident[:],
                        rhs=wg_tw[(t, w)][:, c0 : c0 + r, :].rearrange("p c e -> p (c e)"),
                        start=(gi == 0), stop=(gi == len(ngroups) - 1),
                    )
                qm_c1 = small.tile([P, E], f32, tag="qm_c1")
                nc.scalar.copy(out=qm_c1[:], in_=qm4_psum[:, E : 2 * E])
                qm_c3 = small.tile([P, E], f32, tag="qm_c3")
                nc.scalar.copy(out=qm_c3[:], in_=qm4_psum[:, 3 * E : 4 * E])
                qm_a = small.tile([P, E], f32, tag="qm_a")
                nc.vector.tensor_tensor(out=qm_a[:], in0=qm4_psum[:, 0:E], in1=qm_c1[:], op=OP.add)
                qm_b = small.tile([P, E], f32, tag="qm_b")
                nc.vector.tensor_tensor(out=qm_b[:], in0=qm4_psum[:, 2 * E : 3 * E], in1=qm_c3[:], op=OP.add)
                qm_sb = small.tile([P, E], f32, tag="qm_sb")
                nc.vector.tensor_tensor(out=qm_sb[:], in0=qm_a[:], in1=qm_b[:], op=OP.add)

                qmT_psum = ptr.tile([E, P], f32, tag="mmt", space="PSUM")
                nc.tensor.transpose(out=qmT_psum[:], in_=qm_sb[:], identity=ident_f[:])
                qmT_sb = small.tile([E, P], bf16, tag="qmT_sb")
                nc.scalar.mul(out=qmT_sb[:], in_=qmT_psum[:], mul=1.0 / QW)

                mm1_psum = ptr.tile([E, P], f32, tag="mm", space="PSUM")
                nc.tensor.matmul(out=mm1_psum[:], lhsT=qp_wT_sb[:], rhs=qmT_sb[:], start=True, stop=True)
                qT_f = small.tile([E, P], f32, tag="qT_f")
                nc.scalar.activation(out=qT_f[:], in_=mm1_psum[:], func=AF.Tanh, bias=qp_b_sb[:, 0:1])
                qT_sb = small.tile([E, P], bf16, tag="qT_sb")
                nc.vector.tensor_copy(out=qT_sb[:], in_=qT_f[:])

                q_psum = ptr.tile([P, E], f32, tag="mmt", space="PSUM")
                nc.tensor.transpose(out=q_psum[:], in_=qT_f[:], identity=ident_f[:])
                qhalf_sb = small.tile([P, E], f32, tag="qhalf_sb")
                nc.scalar.mul(out=qhalf_sb[:], in_=q_psum[:], mul=0.5)

                vT_psum = pacc.tile([E, P], f32, tag="vT", space="PSUM")
                for h in range(H):
                    mm2_psum = ptr.tile([E, P], f32, tag="mm", space="PSUM")
                    nc.tensor.matmul(
                        out=mm2_psum[:], lhsT=aq_wT_sb[:, h * E : (h + 1) * E],
                        rhs=qT_sb[:], start=True, stop=True,
                    )
                    t_h = small.tile([E, P], bf16, tag="t_h")
                    nc.scalar.activation(
                        out=t_h[:], in_=mm2_psum[:], func=AF.Tanh, bias=aq_bT_sb[:, h : h + 1]
                    )
                    nc.tensor.matmul(
                        out=vT_psum[:], lhsT=scaledI[:, h * P : (h + 1) * P],
                        rhs=t_h[:], start=(h == 0), stop=(h == H - 1),
                    )
                vT_sb = small.tile([E, P], f32, tag="vT_sb")
                nc.scalar.copy(out=vT_sb[:], in_=vT_psum[:])
                v_psum = ptr.tile([P, E], f32, tag="mmt", space="PSUM")
                nc.tensor.transpose(out=v_psum[:], in_=vT_sb[:], identity=ident_f[:])
                v_sb = small.tile([P, E], bf16, tag="v_sb")
                nc.scalar.copy(out=v_sb[:], in_=v_psum[:])

                iblocks = []
                for w in range(ITEM_WINS):
                    cap = int(item_caps[t][w])
                    if cap:
                        iblocks.append((w, cap))

                # ---- scores: blockwise bought * v-broadcast, reduce over E ----
                scores = small.tile([P, ch], f32, tag="scores")
                loff = 0
                for w, cap in iblocks:
                    prodS = mid.tile([P, cap, E], bf16, tag="prodS")
                    nc.vector.tensor_tensor(
                        out=prodS[:, :, :],
                        in0=bought_tw[(t, w)][:, :, :],
                        in1=v_sb[:, None, :].to_broadcast([P, cap, E]),
                        op=OP.mult,
                    )
                    nc.vector.tensor_reduce(
                        out=scores[:, loff : loff + cap], in_=prodS[:, :, :],
                        axis=mybir.AxisListType.X, op=OP.add,
                    )
                    loff += cap

                negmax = small.tile([P, 1], f32, tag="negmax")
                nc.vector.reduce_max(out=negmax[:], in_=scores[:], axis=mybir.AxisListType.X, negate=True)
                att = small.tile([P, ch], f32, tag="att")
                nc.scalar.activation(out=att[:], in_=scores[:], func=AF.Exp, bias=negmax[:, 0:1])
                attm = small.tile([P, ch], f32, tag="attm")
                nc.vector.tensor_tensor(out=attm[:], in0=att[:], in1=mask_sb[:], op=OP.mult)
                denom = small.tile([P, 1], f32, tag="denom")
                nc.vector.reduce_sum(out=denom[:], in_=attm[:], axis=mybir.AxisListType.X)
                lt01 = small.tile([P, 1], f32, tag="lt01")
                nc.vector.tensor_scalar(out=lt01[:], in0=denom[:], scalar1=1e-7, scalar2=None, op0=OP.is_lt)
                denom2 = small.tile([P, 1], f32, tag="denom2")
                nc.vector.tensor_tensor(out=denom2[:], in0=denom[:], in1=lt01[:], op=OP.add)
                nc.vector.tensor_scalar_mul(out=denom2[:], in0=denom2[:], scalar1=2.0)
                rcp = small.tile([P, 1], f32, tag="rcp")
                nc.vector.reciprocal(out=rcp[:], in_=denom2[:])

                # attm as bf16 for the big product
                attm_b = small.tile([P, ch], bf16, tag="attm_b")
                nc.vector.tensor_copy(out=attm_b[:], in_=attm[:])

                # ---- user: blockwise bought * attm-broadcast + batched matmuls ----
                prodU = big.tile([P, ch, E], bf16, tag="prodU")
                loff = 0
                for w, cap in iblocks:
                    nc.vector.tensor_tensor(
                        out=prodU[:, loff : loff + cap, :],
                        in0=bought_tw[(t, w)][:, :, :],
                        in1=attm_b[:, loff : loff + cap, None].to_broadcast([P, cap, E]),
                        op=OP.mult,
                    )
                    loff += cap
                u4_psum = pacc.tile([P, 4 * E], f32, tag="u4", space="PSUM")
                ngroups = (ch + 3) // 4
                for g in range(ngroups):
                    c0 = g * 4
                    r = min(4, ch - c0)
                    nc.tensor.matmul(
                        out=u4_psum[:, : r * E],
                        lhsT=ident[:],
                        rhs=prodU[:, c0 : c0 + r, :].rearrange("p c e -> p (c e)"),
                        start=(g == 0), stop=(g == ngroups - 1),
                    )
                u_c1 = small.tile([P, E], f32, tag="u_c1")
                nc.scalar.copy(out=u_c1[:], in_=u4_psum[:, E : 2 * E])
                u_c3 = small.tile([P, E], f32, tag="u_c3")
                nc.scalar.copy(out=u_c3[:], in_=u4_psum[:, 3 * E : 4 * E])
                u_a = small.tile([P, E], f32, tag="u_a")
                nc.vector.tensor_tensor(out=u_a[:], in0=u4_psum[:, 0:E], in1=u_c1[:], op=OP.add)
                u_b = small.tile([P, E], f32, tag="u_b")
                nc.vector.tensor_tensor(out=u_b[:], in0=u4_psum[:, 2 * E : 3 * E], in1=u_c3[:], op=OP.add)
                user_sb = small.tile([P, E], f32, tag="user_sb")
                nc.vector.tensor_tensor(out=user_sb[:], in0=u_a[:], in1=u_b[:], op=OP.add)

                out_sb = small.tile([P, E], f32, tag="out_sb")
                nc.vector.scalar_tensor_tensor(
                    out=out_sb[:], in0=user_sb[:], scalar=rcp[:, 0:1],
                    in1=qhalf_sb[:], op0=OP.mult, op1=OP.add,
                )
                nc.sync.dma_start(out=out[b0 : b0 + P, :], in_=out_sb[:])

    nc.finalize()
    return nc


_CACHE = {}


def run(inputs: dict, trace: bool = False, tmpdir: str | None = None):
    from concourse.bass_utils import run_bass_kernel_spmd

    in_maps, shapes, perm = build_host_inputs(inputs)
    key = repr(shapes)
    if key not in _CACHE:
        _CACHE.clear()
        _CACHE[key] = build_nc(shapes)
    nc = _CACHE[key]
    res = run_bass_kernel_spmd(
        nc, in_maps, core_ids=list(range(N_CORES)), trace=trace, tmpdir=tmpdir
    )
    out = np.zeros((B, E), np.float32)
    for c in range(N_CORES):
        out[perm[c]] = np.asarray(res.results[c]["out"], dtype=np.float32)
    return out, res


def kernel(**inputs) -> np.ndarray:
    out, _ = run(inputs, trace=False)
    return out

